# revision 1
# baseline (speedup 1.0000x reference)
"""Trainium2 Bass kernel: single-head causal attention.

Problem: x[4,4096,128]; Q/K/V linear projections (W [in,out] layout, +bias);
scores = QK^T/sqrt(128) with causal mask; softmax; out = P @ V.

Sharding (8 cores = 4 batches x 2): every core runs the SAME program
(SPMD requirement) on different data:
  core (b, h):
    triangle part: queries q in [2048h, 2048h+2048) of batch b attending
        causally to kv rows in the same range (relative causal structure is
        identical for h=0 and h=1).
    rectangle part: queries q in [2048, 4096) of batch b attending to kv rows
        [1024h, 1024h+1024)  (fully valid, no mask, since kv < 2048 <= q).
  Union over both cores of a batch covers the full causal set exactly once.

Softmax is computed WITHOUT max subtraction (scores are ~N(0,1) by
construction: Wq is pre-scaled by 1/sqrt(128) on host, so exp never
overflows), which makes the cross-core merge linear: the host sums
unnormalized outputs o and denominators l, then divides.

Bias handling:
  - bk drops out of softmax entirely (adds a per-query constant to scores).
  - bq is pre-scaled on host and added to Q^T during the PSUM->SBUF copy
    (per-partition scalar add on the vector engine).
  - bv is added on the host after normalization (rows of P sum to 1).

Matmuls run in float32r (TF32-like: fp32 storage, 11-bit mantissa, full PE
rate at moving free dim >= 256). The BIR verifier requires every producer of
an f32r matmul operand to emit f32r (hardware rounds on write); host-side
inputs are pre-rounded with the exact RNE-to-11-bits rule.

Device layouts (per core):
  xTq [128,4096]  x^T columns for this core's 4096 query slots (tri|rect)
  xTk [128,3072]  x^T columns for kv rows (tri 2048 | rect 1024)
  QT = (x@Wq')^T + bq'  [128(e), 4096(q)]   (e on partitions)
  KT = (x@Wk)^T         [128(e), 3072(k)]
  V  = x@Wv    as 24 tiles [128(kv row), 128(e)] packed in [128, 3072]
  Scores are computed TRANSPOSED: ST[k, q] = K Q^T (PSUM), masked on
  diagonal tiles, exp'd on the scalar engine into P~T [k, q] (SBUF).
  AV:  oT[e, q] += V_t^T-matmul-P~T   (accumulated in PSUM over kv tiles)
  l:   l[q]    += ones-matmul-P~T     (PE is the only partition reducer)
Outputs: oT [128, 4096] (transposed, unnormalized), lv [8,512] (denominators
per 512-query chunk). Host transposes, merges, normalizes, adds bv.
"""

import math
import sys

import numpy as np

sys.path.insert(0, "/opt/trn_rl_repo")

import concourse.bass as bass  # noqa: E402
import concourse.mybir as mybir  # noqa: E402
from concourse.tile import TileContext  # noqa: E402

B, T, D = 4, 4096, 128
HALF = T // 2          # 2048 queries per triangle
NCHUNK = 8             # 8 chunks of 512 query slots per core (4 tri + 4 rect)
CHUNK = 512
KV_TRI_TILES = 16      # triangle kv tiles (2048 rows)
KV_RECT_TILES = 8      # rectangle kv tiles (1024 rows)
KV_TILES = KV_TRI_TILES + KV_RECT_TILES          # 24 tiles = 3072 kv rows
NEG = -1.0e5           # additive mask value; exp(NEG) == 0.0 in fp32

F32 = mybir.dt.float32
F32R = mybir.dt.float32r


def round_f32r(a):
    """Exact fp32 -> fp32r rounding (RNE to 11 mantissa bits), matching
    walrus fp32_to_fp32r."""
    u = np.ascontiguousarray(a, np.float32).view(np.uint32)
    add = np.uint32(0x7FF) + ((u >> np.uint32(12)) & np.uint32(1))
    return ((u + add) & np.uint32(0xFFFFF000)).view(np.float32)


def build_nc(legalize=True):
    nc = bass.Bass()

    xtq_d = nc.declare_dram_parameter("xTq", [D, T], F32R, isOutput=False)
    xtk_d = nc.declare_dram_parameter("xTk", [D, KV_TILES * 128], F32R, isOutput=False)
    wq_d = nc.declare_dram_parameter("Wqs", [D, D], F32R, isOutput=False)
    wk_d = nc.declare_dram_parameter("Wk", [D, D], F32R, isOutput=False)
    wv_d = nc.declare_dram_parameter("Wv", [D, D], F32R, isOutput=False)
    bq_d = nc.declare_dram_parameter("bqs", [D], F32, isOutput=False)
    msk_d = nc.declare_dram_parameter("msk", [4, D, CHUNK], F32R, isOutput=False)
    ident_d = nc.declare_dram_parameter("ident", [D, D], F32R, isOutput=False)
    ones_d = nc.declare_dram_parameter("ones", [D, 1], F32R, isOutput=False)

    ot_d = nc.declare_dram_parameter("oT", [D, T], F32, isOutput=True)
    lv_d = nc.declare_dram_parameter("lv", [NCHUNK, CHUNK], F32, isOutput=True)

    with TileContext(nc) as tc:
        with (
            tc.tile_pool(name="big", bufs=1) as big,
            tc.tile_pool(name="small", bufs=1) as small,
        ):
            # ---- resident SBUF tensors: first-consumed DMAs first (the
            # V projection needs wv + xtk chunk 0 before anything else) ----
            wv = small.tile([D, D], F32R)
            nc.sync.dma_start(out=wv, in_=wv_d[:, :])
            xtk = big.tile([D, KV_TILES * 128], F32R)
            nc.sync.dma_start(out=xtk[:, 0:CHUNK], in_=xtk_d[:, 0:CHUNK])
            wk = small.tile([D, D], F32R)
            nc.sync.dma_start(out=wk, in_=wk_d[:, :])
            wq = small.tile([D, D], F32R)
            nc.sync.dma_start(out=wq, in_=wq_d[:, :])
            bq = small.tile([D, 1], F32)
            nc.sync.dma_start(out=bq, in_=bq_d[:].unsqueeze(1))
            ones = small.tile([D, 1], F32R)
            nc.sync.dma_start(out=ones, in_=ones_d[:, :])
            for j in range(1, KV_TILES * 128 // CHUNK):
                sl = slice(j * CHUNK, (j + 1) * CHUNK)
                nc.sync.dma_start(out=xtk[:, sl], in_=xtk_d[:, sl])
            xtq = big.tile([D, T], F32R)
            for j in range(T // 1024):
                sl = slice(j * 1024, (j + 1) * 1024)
                nc.sync.dma_start(out=xtq[:, sl], in_=xtq_d[:, sl])
            ident = small.tile([D, D], F32R)
            nc.sync.dma_start(out=ident, in_=ident_d[:, :])
            msk = big.tile([D, 4 * CHUNK], F32R)
            nc.sync.dma_start(
                out=msk.rearrange("p (m q) -> p m q", m=4),
                in_=msk_d[:, :, :].transpose([1, 0, 2]),
            )

            qt = big.tile([D, T], F32R)               # Q^T (scaled, biased)
            kt = big.tile([D, KV_TILES * 128], F32R)  # K^T
            vsb = big.tile([D, KV_TILES * 128], F32R)  # V tiles [kvrow, e]

            # The ST pool is opened FIRST so the stack allocator gives it
            # PSUM banks the projection phase never touches: the first
            # attention score matmuls then carry no release deps from the
            # projection pools and overlap the projection tail on the PE.
            stp_cm = tc.tile_pool(name="stp", bufs=2, space="PSUM")
            stp = stp_cm.__enter__()
            # ---- projections (order: V, K, Q so the DVE tick PE waits on
            # for qt also covers vsb/kt; "touch" matmuls absorb each DMA
            # semaphore into PE's clock first, because the fused-weight-load
            # fp32r matmul instruction supports only ONE sync wait) ----
            with (
                tc.tile_pool(name="ppsum", bufs=1, space="PSUM")) as ppsum:
                # (the former "touch" matmuls that absorbed DMA semaphores
                # into PE's clock are gone: the post-Tile wait legalizer
                # handles multi-wait instructions directly, and dropping
                # them frees their PSUM bank for a 4-deep projection
                # rotation plus ~2us of PE dispatch)

                # Pool-recycled PSUM tiles hand every accessor of the new
                # tile the old tile's full release deps (PE write + DVE read)
                # - 2 sync waits, over the fused-weight-load fp32r matmul
                # limit of 1. A single persistent 3-bank tile with manual
                # region rotation keeps deps intra-tile: same-engine WAW is
                # program-order (no sem), so each matmul carries only the
                # DVE WAR wait.
                pps = [ppsum.tile([D, CHUNK], F32, name=f"pps{s}")
                       for s in range(4)]
                nps = [0]

                def proj_ps():
                    s = nps[0] % 4
                    nps[0] += 1
                    return pps[s], s

                for g in range(KV_TILES // 4):     # V: 24 tiles, batched 4/bank
                    ps, s = proj_ps()
                    for jj in range(4):
                        t = 4 * g + jj
                        nc.tensor.matmul(
                            ps[:, jj * 128:(jj + 1) * 128],
                            xtk[:, t * 128:(t + 1) * 128], wv,
                            start=True, stop=True, skip_group_check=True,
                        )
                    if g % 2 == 0:
                        nc.vector.tensor_copy(
                            vsb[:, g * CHUNK:(g + 1) * CHUNK], ps)
                    else:
                        nc.scalar.copy(vsb[:, g * CHUNK:(g + 1) * CHUNK], ps)
                for j in range(KV_TILES * 128 // CHUNK):   # K^T: 6 chunks
                    ps, s = proj_ps()
                    nc.tensor.matmul(
                        ps, wk, xtk[:, j * CHUNK:(j + 1) * CHUNK],
                        start=True, stop=True, skip_group_check=True,
                    )
                    if j % 2 == 0:
                        nc.vector.tensor_copy(
                            kt[:, j * CHUNK:(j + 1) * CHUNK], ps)
                    else:
                        nc.scalar.copy(kt[:, j * CHUNK:(j + 1) * CHUNK], ps)
                for j in range(T // CHUNK):        # Q^T: 8 chunks
                    ps, s = proj_ps()
                    nc.tensor.matmul(
                        ps, wq, xtq[:, j * CHUNK:(j + 1) * CHUNK],
                        start=True, stop=True, skip_group_check=True,
                    )
                    if j % 2 == 0:
                        nc.vector.tensor_scalar_add(
                            qt[:, j * CHUNK:(j + 1) * CHUNK], ps, bq)
                    else:
                        nc.scalar.activation(
                            qt[:, j * CHUNK:(j + 1) * CHUNK], ps,
                            mybir.ActivationFunctionType.Identity, bias=bq)
                # final pump: absorb the last DVE copies before attention

            # ---- attention: 8 chunks, kv-tile pairs, software-pipelined ----
            # chunk c covers query slots [512c, 512c+512).
            # tri chunks (0-3): kv tiles 0..4c+3; rect chunks (4-7): 16..23.
            # Pairs are processed in REVERSE kv order so the diagonal
            # (masked) pairs land at chunk starts, where the previous
            # chunk's AV/l matmuls hide the mask-add + exp latency.
            # The AV+l matmuls of unit u are emitted after ST/exp of unit
            # u+1 (skew-1 software pipeline) so PE never waits on ACT.
            # Tri chunks: the 4 diagonal tiles first in ASCENDING m order
            # (so the first AV/l matmul of the chunk covers the full column
            # range with start=True and later sliced matmuls only ever
            # accumulate onto initialized columns), then the full tiles.
            chunk_ts = [list(range(4 * c, 4 * c + 4)) +
                        list(range(0, 4 * c))[::-1] for c in range(4)] + \
                       [list(range(16, 24))[::-1] for _ in range(4)]
            units = []
            for c, ts in enumerate(chunk_ts):
                pairs = [ts[i:i + 2] for i in range(0, len(ts), 2)]
                for pi, pair in enumerate(pairs):
                    units.append((c, ts, pair, pi == len(pairs) - 1))
            with (
                tc.tile_pool(name="op", bufs=2, space="PSUM") as op,
                tc.tile_pool(name="lp", bufs=2, space="PSUM") as lp,
                tc.tile_pool(name="ptp", bufs=1) as ptp,
                tc.tile_pool(name="osb", bufs=8) as osb,
                tc.tile_pool(name="lsb", bufs=8) as lsb,
            ):
                pts = [ptp.tile([D, 2 * CHUNK], F32R, name=f"pt{i}")
                       for i in range(3)]
                npt = [0]
                acc = {}                # chunk -> (po, pl)
                pending = None          # (c, ts, pair, is_last, pt)
                epiq = []               # delayed epilogues [(c, po, pl)]

                def emit_epilogue():
                    c, po, pl = epiq.pop(0)
                    # epilogue copies on ACT (scalar): the PSUM-slot WAR
                    # dependency of a later chunk's first AV matmul then
                    # consolidates onto the ACT semaphore (1-wait limit).
                    # Delayed one pipeline unit so these ACT ops never sit
                    # between an ST matmul and the exp PE is waiting for.
                    qsl = slice(c * CHUNK, (c + 1) * CHUNK)
                    ob = osb.tile([D, CHUNK], F32, tag="ob", name="ob")
                    nc.vector.tensor_copy(ob, po)
                    nc.sync.dma_start(out=ot_d[:, qsl], in_=ob)
                    lb = lsb.tile([1, CHUNK], F32, tag="lb", name="lb")
                    nc.vector.tensor_copy(lb, pl)
                    nc.sync.dma_start(out=lv_d[c:c + 1, :], in_=lb)

                def emit_av(pend):
                    c, ts, pair, is_last, pt, los = pend
                    if c not in acc:
                        acc[c] = (
                            op.tile([D, CHUNK], F32, tag="po", name="po"),
                            lp.tile([1, CHUNK], F32, tag="pl", name="pl"),
                        )
                    po, pl = acc[c]
                    qsl = slice(c * CHUNK, (c + 1) * CHUNK)
                    for i, t in enumerate(pair):
                        lo = los[i]
                        ptc = pt[:, i * CHUNK + lo:(i + 1) * CHUNK]
                        nc.tensor.matmul(
                            po[:, lo:], vsb[:, t * 128:(t + 1) * 128], ptc,
                            start=(t == ts[0]), stop=(t == ts[-1]),
                            skip_group_check=True,
                        )
                        nc.tensor.matmul(
                            pl[0:1, lo:], ones, ptc,
                            start=(t == ts[0]), stop=(t == ts[-1]),
                            skip_group_check=True,
                        )
                    if is_last:
                        epiq.append((c, po, pl))
                        del acc[c]

                for c, ts, pair, is_last in units:
                    if epiq:
                        emit_epilogue()
                    # Diagonal sub-tile m: every score column q' < 128m is
                    # fully masked (q' < 128m <= 128m + k for all k), so the
                    # ST / mask / exp / AV / l work all skip that prefix.
                    # Within the remaining window only the 128-column band
                    # [128m, 128(m+1)) needs the staircase mask.
                    los = [128 * (t - 4 * c) if c < 4 and t >= 4 * c else 0
                           for t in pair]
                    st = stp.tile([D, 2 * CHUNK], F32, tag="st", name="st")
                    for i, t in enumerate(pair):
                        lo = los[i]
                        nc.tensor.matmul(
                            st[:, i * CHUNK + lo:(i + 1) * CHUNK],
                            kt[:, t * 128:(t + 1) * 128],
                            qt[:, c * CHUNK + lo:(c + 1) * CHUNK],
                            start=True, stop=True, skip_group_check=True,
                        )
                        if c < 4 and t >= 4 * c:
                            m = t - 4 * c
                            nc.tensor.matmul(
                                st[:, i * CHUNK + lo:i * CHUNK + lo + 128],
                                ident,
                                msk[:, m * CHUNK + lo:m * CHUNK + lo + 128],
                                start=False, stop=True, skip_group_check=True,
                            )
                    pt = pts[npt[0] % 3]
                    npt[0] += 1
                    if len(pair) == 2 and los[1] > 0:
                        # sliced halves with an uninitialized gap: exp each
                        # half's valid window separately
                        nc.scalar.activation(
                            pt[:, los[0]:CHUNK], st[:, los[0]:CHUNK],
                            mybir.ActivationFunctionType.Exp,
                        )
                        nc.scalar.activation(
                            pt[:, CHUNK + los[1]:], st[:, CHUNK + los[1]:],
                            mybir.ActivationFunctionType.Exp,
                        )
                    else:
                        nc.scalar.activation(
                            pt[:, los[0]:], st[:, los[0]:],
                            mybir.ActivationFunctionType.Exp,
                        )
                    prev, pending = pending, (c, ts, pair, is_last, pt, los)
                    if prev is not None:
                        emit_av(prev)
                emit_av(pending)
                while epiq:
                    emit_epilogue()
            stp_cm.__exit__(None, None, None)

    if legalize:
        _legalize_multiwaits(nc)
    nc.finalize()
    return nc


def _legalize_multiwaits(nc):
    """Hardware instruction structs in this walrus build accept at most ONE
    sync wait. For any instruction left with >= 2 waits after Tile's sem
    assignment, move all but the last wait onto single-wait same-engine
    NoOps inserted right before it. Engines execute in order, so waiting
    earlier on the same engine preserves semantics exactly.
    """
    for fn in nc.m.functions:
        for blk in fn.blocks:
            insts = blk.instructions
            out = []
            for inst in insts:
                si = inst.sync_info
                if si is not None and si.on_wait and len(si.on_wait) >= 2:
                    waits = list(si.on_wait)
                    for w in waits[:-1]:
                        out.append(mybir.InstNoOp(
                            name=nc.get_next_instruction_name(),
                            engine=inst.engine,
                            bass_nofuse=True,
                            sync_info=mybir.SyncInfo(
                                on_wait=[w], on_update=[]),
                        ))
                    inst.sync_info = mybir.SyncInfo(
                        on_wait=[waits[-1]],
                        on_update=list(si.on_update or []))
                out.append(inst)
            insts[:] = out


_NC_CACHE = {}


def get_nc(legalize=True):
    key = ("nc", legalize)
    if key not in _NC_CACHE:
        _NC_CACHE[key] = build_nc(legalize)
    return _NC_CACHE[key]


def make_core_inputs(x, Wq, bq, Wk, bk, Wv, bv):
    """Per-core input maps (host-side sharding). bk is dropped (softmax
    invariance); bv is applied on the host. f32r-consumed inputs are
    pre-rounded to match the hardware's assumed rounding."""
    s = 1.0 / math.sqrt(D)
    wq_s = round_f32r(np.asarray(Wq, np.float32) * s)
    bq_s = (np.asarray(bq, np.float32) * s).astype(np.float32)
    wk = round_f32r(np.asarray(Wk, np.float32))
    wv = round_f32r(np.asarray(Wv, np.float32))

    # diagonal masks: msk[m][k, q'] = 0 if q' >= 128*m + k else NEG
    qp = np.arange(CHUNK)[None, :]
    kk = np.arange(128)[:, None]
    msk = round_f32r(np.stack(
        [np.where(qp >= 128 * m + kk, 0.0, NEG) for m in range(4)]
    ).astype(np.float32)).reshape(4, D, CHUNK)
    ident = np.eye(D, dtype=np.float32)

    ones = np.ones((D, 1), np.float32)

    x = np.asarray(x, dtype=np.float32)
    in_maps = []
    for core in range(8):
        b, h = core // 2, core % 2
        xb = x[b]                                   # [4096, 128]
        tri = xb[h * HALF:(h + 1) * HALF]           # [2048, 128]
        rect_q = xb[HALF:]                          # [2048, 128]
        rect_kv = xb[h * 1024:(h + 1) * 1024]       # [1024, 128]
        xtq = round_f32r(np.ascontiguousarray(
            np.concatenate([tri, rect_q], axis=0).T))     # [128, 4096]
        xtk = round_f32r(np.ascontiguousarray(
            np.concatenate([tri, rect_kv], axis=0).T))    # [128, 3072]
        in_maps.append({
            "xTq": xtq, "xTk": xtk, "Wqs": wq_s, "Wk": wk, "Wv": wv,
            "bqs": bq_s, "msk": msk, "ones": ones, "ident": ident,
        })
    return in_maps


def merge_outputs(results, bv):
    """Gather per-core (oT, lv) into the full [B, T, D] output."""
    bv = np.asarray(bv, dtype=np.float32)
    out = np.empty((B, T, D), np.float32)
    for b in range(B):
        lo, hi = results[2 * b], results[2 * b + 1]
        O = np.zeros((T, D), np.float64)
        L = np.zeros(T, np.float64)
        O[:HALF] += lo["oT"][:, :HALF].T
        L[:HALF] += lo["lv"][0:4].ravel()
        O[HALF:] += hi["oT"][:, :HALF].T
        L[HALF:] += hi["lv"][0:4].ravel()
        O[HALF:] += lo["oT"][:, HALF:].T
        L[HALF:] += lo["lv"][4:8].ravel()
        O[HALF:] += hi["oT"][:, HALF:].T
        L[HALF:] += hi["lv"][4:8].ravel()
        out[b] = (O / L[:, None]).astype(np.float32) + bv
    return out


def run_per_core(nc, in_maps, threads=True):
    """Run the same single-core program on each NeuronCore with its own
    inputs. The multi-core shard_map path in run_bass_via_pjrt stalls under
    this container's axon tunnel; independent single-device dispatches work
    (the cores share no collectives, so per-core dispatch is equivalent)."""
    import jax
    from concourse import bass2jax

    devices = jax.devices()[:len(in_maps)]

    def one(i):
        with jax.default_device(devices[i]):
            return bass2jax.run_bass_via_pjrt(nc, [in_maps[i]], n_cores=1)[0]

    if threads:
        from concurrent.futures import ThreadPoolExecutor
        # warm the compile cache once to avoid 8 racing neuronxcc compiles
        first = one(0)
        with ThreadPoolExecutor(max_workers=7) as ex:
            rest = list(ex.map(one, range(1, len(in_maps))))
        return [first] + rest
    return [one(i) for i in range(len(in_maps))]


def kernel(x, Wq, bq, Wk, bk, Wv, bv, _trace=False):
    from concourse.bass_utils import axon_active, run_bass_kernel_spmd

    nc = get_nc()
    in_maps = make_core_inputs(x, Wq, bq, Wk, bk, Wv, bv)
    if axon_active():
        # This container tunnels devices through axon; the 8-device
        # shard_map dispatch stalls there, so dispatch per-core.
        results = run_per_core(nc, in_maps)
    else:
        # Native /dev/neuron*: the production NrtSession path.
        res = run_bass_kernel_spmd(nc, in_maps, list(range(8)), trace=_trace)
        kernel.last_result = res
        results = res.results
    out = merge_outputs(results, bv)
    return out



# revision 20
# speedup vs baseline: 1.2408x; 1.2408x over previous
"""Trainium2 Bass kernel: single-head causal attention (fp8-DoubleRow design).

Problem: x[4,4096,128]; Q/K/V linear projections (W [in,out] layout, +bias);
scores = QK^T/sqrt(128) with causal mask; softmax; out = P @ V.

Sharding (8 cores = 4 batches x 2), same as v1:
  core (b, h):
    triangle: queries [2048h, 2048h+2048) of batch b attending causally to
        the same kv range (chunks 0-3, kv tiles 0-15).
    rectangle: queries [2048, 4096) attending to kv [1024h, 1024h+1024)
        (chunks 4-7, kv tiles 16-23; no mask).
  Union over both cores of a batch covers the causal set exactly once.
Softmax without max-subtraction; host merges unnormalized (oT, lv).

v2 speed design:
  - x deduplicated ([tri | rect_q | rect_kv] = 5120 cols) and shipped BF16.
  - All projection / chunk-0 matmuls in bf16 (1.0 cycles/row at any width).
  - Scores are computed PRE-SCALED by 8/ln2 (folded into Wq/bq): st' = s*11.54.
  - Chunks 1-7: "exp" is the fp8 bit trick: int8 bits = trunc(max(st'+B, 0))
    IS the bitpattern of fp8e4(e^s / 4) up to mantissa interpolation (~6%).
    One DVE tensor_scalar (add,max) or ACT Relu per pair tile produces P
    directly in fp8; masked elements (st' - 99840) clamp to bits 0 = +0.0.
  - ST/AV/l matmuls for chunks 1-7 run as fp8 DoubleRow (0.5 cycles/row):
      ST: e-dim packed [64,2]: st = sum_j ktp[:,j,:].T @ qtp[:,j,:]
      AV: kv-pair packed: po += sum_j V[:,j,:].T @ P[:,j,:]
      l:  ones [128,2,1]: pl += sum_j 1.T @ P[:,j,:]
    ktp/qtp [64,2,*] fp8 layouts made by SBUF->SBUF repartition DMAs.
  - Chunk 0 (small-m queries; accuracy-critical) runs the bf16 path with a
    real ACT exp (scale=1/11.54, bias=-ln4 keeps the P scale uniform).
  - l rows of all 8 chunks accumulate into ONE PSUM bank at partition
    offsets 0..7; a single epilogue copy + DMA writes lv [8,512].
The common P scale (e^s / 4) cancels in the host's O/l division.
"""

import math
import sys

import numpy as np
import ml_dtypes

sys.path.insert(0, "/opt/trn_rl_repo")

import concourse.bass as bass  # noqa: E402
import concourse.mybir as mybir  # noqa: E402
from concourse.tile import TileContext  # noqa: E402

B, T, D = 4, 4096, 128
HALF = T // 2
NCHUNK = 8
CHUNK = 512
KV_TILES = 24          # 16 tri + 8 rect
XCOLS = 5120           # tri 2048 | rect_q 2048 | rect_kv 1024
NEG = -99840.0         # additive mask value (bf16-exact)

F32 = mybir.dt.float32
BF16 = mybir.dt.bfloat16
F8 = mybir.dt.float8e4
I8 = mybir.dt.int8
DR = mybir.MatmulPerfMode.DoubleRow

BITS_SCALE = 8.0 / math.log(2.0)       # 11.5416, folded into Wq/bq
BITS_B = 40.05                         # bits = trunc(st' + B); ~mean-centered
EXP_SCALE = 1.0 / BITS_SCALE
EXP_BIAS = -2.0 * math.log(2.0)        # e^s / 4 to match the fp8 P scale


def kv_col(t):
    """xT column offset of kv tile t."""
    return t * 128 if t < 16 else 4096 + (t - 16) * 128


def build_nc(legalize=True):
    nc = bass.Bass()

    xt_d = nc.declare_dram_parameter("xT", [D, XCOLS], BF16, isOutput=False)
    wq_d = nc.declare_dram_parameter("Wqs", [D, D], BF16, isOutput=False)
    wk_d = nc.declare_dram_parameter("Wk", [D, D], BF16, isOutput=False)
    wv_d = nc.declare_dram_parameter("Wv", [D, D], BF16, isOutput=False)
    bq_d = nc.declare_dram_parameter("bqs", [D], F32, isOutput=False)
    msk_d = nc.declare_dram_parameter("msk", [4, D, CHUNK], BF16, isOutput=False)
    ident_d = nc.declare_dram_parameter("ident", [D, D], BF16, isOutput=False)

    ot_d = nc.declare_dram_parameter("oT", [D, T], F32, isOutput=True)
    lv_d = nc.declare_dram_parameter("lv", [NCHUNK, CHUNK], F32, isOutput=True)

    with TileContext(nc) as tc:
        with (
            tc.tile_pool(name="big", bufs=1) as big,
            tc.tile_pool(name="small", bufs=1) as small,
        ):
            # ---- resident SBUF tensors, first-consumed DMAs first ----
            wk = small.tile([D, D], BF16)
            nc.sync.dma_start(out=wk, in_=wk_d[:, :])
            wq = small.tile([D, D], BF16)
            nc.sync.dma_start(out=wq, in_=wq_d[:, :])
            wv = small.tile([D, D], BF16)
            nc.sync.dma_start(out=wv, in_=wv_d[:, :])
            bq = small.tile([D, 1], F32)
            nc.sync.dma_start(out=bq, in_=bq_d[:].unsqueeze(1))
            xt = big.tile([D, XCOLS], BF16)
            # kv columns first (tri 0:2048, rect_kv 4096:5120), queries after
            for lo, hi in ((0, 1024), (1024, 2048), (4096, 5120),
                           (2048, 3072), (3072, 4096)):
                nc.sync.dma_start(out=xt[:, lo:hi], in_=xt_d[:, lo:hi])
            ident = small.tile([D, D], BF16)
            nc.sync.dma_start(out=ident, in_=ident_d[:, :])
            msk = big.tile([D, 4 * CHUNK], BF16)
            nc.sync.dma_start(
                out=msk.rearrange("p (m q) -> p m q", m=4),
                in_=msk_d[:, :, :].transpose([1, 0, 2]),
            )

            # const tiles
            ones16 = small.tile([D, 1], BF16)
            nc.gpsimd.memset(ones16, 1.0)
            # DoubleRow ldweights needs >=32 out columns; 32 identical
            # l rows cost the same (PE charges output free size only)
            ones8 = small.tile([D, 2, 32], F8)
            nc.gpsimd.memset(ones8, 1.0)
            bias_e = small.tile([D, 1], F32)
            nc.gpsimd.memset(bias_e, EXP_BIAS)
            bias_b = small.tile([D, 1], F32)
            nc.gpsimd.memset(bias_b, BITS_B)

            # attention-side SBUF
            kt16 = small.tile([D, 4 * 128], BF16)    # K^T tiles 0-3 (chunk 0)
            qt16 = small.tile([D, CHUNK], BF16)      # Q^T chunk 0
            vs16 = small.tile([D, 4 * 128], BF16)    # V tiles 0-3 [kv,e]
            k8f = big.tile([D, KV_TILES * 128], F8)  # K^T flat fp8
            q8f = big.tile([D, T], F8)               # Q^T flat fp8
            vf8 = big.tile([D, KV_TILES * 128], F8)  # V tiles [kv,e] fp8
            ktp = big.tile([64, 2, KV_TILES * 128], F8)  # K^T e-pair-packed
            qtp = big.tile([64, 2, T], F8)               # Q^T e-pair-packed

            # ST pool first so it gets PSUM banks the projections never
            # touch (overlap of first ST with projection tail).
            stp_cm = tc.tile_pool(name="stp", bufs=2, space="PSUM")
            stp = stp_cm.__enter__()

            # ---- projections ----
            with tc.tile_pool(name="ppsum", bufs=1, space="PSUM") as ppsum:
                pps = [ppsum.tile([D, CHUNK], F32, name=f"pps{s}")
                       for s in range(4)]
                nps = [0]

                def proj_ps():
                    s = nps[0] % 4
                    nps[0] += 1
                    return pps[s]

                ncopy = [0]

                def copy_eng():
                    ncopy[0] += 1
                    return nc.vector if ncopy[0] % 2 == 0 else nc.scalar

                def copy_f8(dst, src, bias=None):
                    e = copy_eng()
                    if bias is None:
                        if e is nc.vector:
                            nc.vector.tensor_copy(dst, src)
                        else:
                            nc.scalar.copy(dst, src)
                    else:
                        if e is nc.vector:
                            nc.vector.tensor_scalar_add(dst, src, bias)
                        else:
                            nc.scalar.activation(
                                dst, src,
                                mybir.ActivationFunctionType.Identity,
                                bias=bias)

                # K tiles 0-3 -> kt16 + k8f[0:512]
                ps = proj_ps()
                nc.tensor.matmul(ps, wk, xt[:, 0:CHUNK],
                                 start=True, stop=True, skip_group_check=True)
                nc.vector.tensor_copy(kt16, ps)
                nc.scalar.copy(k8f[:, 0:CHUNK], ps)
                # Q chunk 0 -> qt16 (bf16 path only)
                ps = proj_ps()
                nc.tensor.matmul(ps, wq, xt[:, 0:CHUNK],
                                 start=True, stop=True, skip_group_check=True)
                nc.vector.tensor_scalar_add(qt16, ps, bq)
                # V tiles 0-3 -> vs16 + vf8 pairs 0-1
                ps = proj_ps()
                for jj in range(4):
                    nc.tensor.matmul(
                        ps[:, jj * 128:(jj + 1) * 128],
                        xt[:, jj * 128:(jj + 1) * 128], wv,
                        start=True, stop=True, skip_group_check=True)
                nc.vector.tensor_copy(vs16, ps)
                nc.scalar.copy(vf8[:, 0:CHUNK], ps)

                # remaining K chunks (tiles 4-23)
                for j in range(1, 6):
                    ps = proj_ps()
                    c0 = CHUNK * j
                    xsl = slice(kv_col(4 * j), kv_col(4 * j) + CHUNK)
                    nc.tensor.matmul(ps, wk, xt[:, xsl],
                                     start=True, stop=True,
                                     skip_group_check=True)
                    copy_f8(k8f[:, c0:c0 + CHUNK], ps)
                # remaining V chunks (tiles 4-23)
                for j in range(1, 6):
                    ps = proj_ps()
                    c0 = CHUNK * j
                    for jj in range(4):
                        t = 4 * j + jj
                        nc.tensor.matmul(
                            ps[:, jj * 128:(jj + 1) * 128],
                            xt[:, kv_col(t):kv_col(t) + 128], wv,
                            start=True, stop=True, skip_group_check=True)
                    copy_f8(vf8[:, c0:c0 + CHUNK], ps)
                # remaining Q chunks 1-7
                for c in range(1, 8):
                    ps = proj_ps()
                    nc.tensor.matmul(ps, wq, xt[:, c * CHUNK:(c + 1) * CHUNK],
                                     start=True, stop=True,
                                     skip_group_check=True)
                    copy_f8(q8f[:, c * CHUNK:(c + 1) * CHUNK], ps, bias=bq)

            # ---- repartition DMAs: [128, n] fp8 -> [64, 2, n] ----
            # Issued from the (otherwise idle) Pool engine's SWDGE so they
            # don't queue behind SP's input DMAs. K tiles 0-7 + Q chunk 1
            # first (chunk-1 needs them), then the rest.
            for lo, hi in ((0, 1024), (1024, 3072)):
                nc.gpsimd.dma_start(out=ktp[:, 0, lo:hi], in_=k8f[0:64, lo:hi])
                nc.gpsimd.dma_start(out=ktp[:, 1, lo:hi], in_=k8f[64:128, lo:hi])
            for lo, hi in ((512, 2048), (2048, 4096)):
                nc.gpsimd.dma_start(out=qtp[:, 0, lo:hi], in_=q8f[0:64, lo:hi])
                nc.gpsimd.dma_start(out=qtp[:, 1, lo:hi], in_=q8f[64:128, lo:hi])

            # ---- attention ----
            # units: (c, pair_tiles, lo, kind); chunk c queries [512c,512c+512)
            # tri chunk c: diag pairs (4c+0,4c+1) lo=0 and (4c+2,4c+3) lo=256
            #   then off-diag pairs (0,1)..(4c-2,4c-1) lo=0
            # rect chunk: pairs (16,17)..(22,23) lo=0
            # All chunk>=1 pairs are full-width (lo=0): masked prefixes of
            # diagonal tiles become exact +0.0 after mask+trick clamp, so
            # full-width AV/l matmuls stay correct (sliced 1-partition DR
            # outputs fail the walrus ISA check).
            units = []
            for c in range(4):
                ts = [((4 * c, 4 * c + 1), 0, "diag"),
                      ((4 * c + 2, 4 * c + 3), 0, "diag")]
                ts += [((2 * p, 2 * p + 1), 0, "off") for p in range(2 * c)]
                for i, (pair, lo, kind) in enumerate(ts):
                    units.append((c, pair, lo, kind, i == len(ts) - 1))
            for c in range(4, 8):
                for p in range(4):
                    units.append((c, (16 + 2 * p, 17 + 2 * p), 0, "off",
                                  p == 3))

            with (
                tc.tile_pool(name="op", bufs=2, space="PSUM") as op,
                tc.tile_pool(name="lp", bufs=2, space="PSUM") as lp,
                tc.tile_pool(name="ptp", bufs=1) as ptp,
                tc.tile_pool(name="osb", bufs=8) as osb,
            ):
                lb = osb.tile([1, NCHUNK * CHUNK], F32, name="lb")
                pt8s = [ptp.tile([D, 2, CHUNK], I8, name=f"pt8_{i}")
                        for i in range(3)]
                pt16s = [ptp.tile([D, 2, CHUNK], BF16, name=f"pt16_{i}")
                         for i in range(2)]
                npt = [0]
                npt16 = [0]
                ntrick = [0]
                acc = {}            # chunk -> (po, pl) psum tiles
                pending = None
                epiq = []
                nepi = [0]

                def emit_epilogue():
                    c, po, pl = epiq.pop(0)
                    qsl = slice(c * CHUNK, (c + 1) * CHUNK)
                    ob = osb.tile([D, CHUNK], F32, tag="ob", name="ob")
                    nepi[0] += 1
                    if nepi[0] % 2 == 0:
                        nc.vector.tensor_copy(ob, po)
                        nc.scalar.copy(lb[:, c * CHUNK:(c + 1) * CHUNK], pl[0:1, :])
                    else:
                        nc.scalar.copy(ob, po)
                        nc.vector.tensor_copy(lb[:, c * CHUNK:(c + 1) * CHUNK], pl[0:1, :])
                    nc.sync.dma_start(out=ot_d[:, qsl], in_=ob)

                def emit_av(pend):
                    c, pair, lo, kind, is_last, pt = pend
                    first = c not in acc
                    if first:
                        acc[c] = (
                            op.tile([D, CHUNK], F32, tag="po", name="po"),
                            lp.tile([32, CHUNK], F32, tag="pl", name="pl"),
                        )
                    po, pl = acc[c]
                    if c == 0:
                        # bf16 per-tile AV + l, full width (masked cols are 0)
                        for i, t in enumerate(pair):
                            nc.tensor.matmul(
                                po, vs16[:, t * 128:(t + 1) * 128],
                                pt[:, i, :],
                                start=(t == 0), stop=(t == 3),
                                skip_group_check=True)
                            nc.tensor.matmul(
                                pl[0:1, :], ones16, pt[:, i, :],
                                start=(t == 0), stop=(t == 3),
                                skip_group_check=True)
                    else:
                        t0 = pair[0]
                        p8 = pt[:, :, lo:].bitcast(F8)
                        nc.tensor.matmul(
                            po[:, lo:],
                            vf8[:, t0 * 128:(t0 + 2) * 128].rearrange(
                                "p (two e) -> p two e", two=2),
                            p8,
                            start=first, stop=is_last,
                            perf_mode=DR, skip_group_check=True)
                        nc.tensor.matmul(
                            pl[0:32, :], ones8, p8,
                            start=first, stop=is_last,
                            perf_mode=DR, skip_group_check=True)
                    if is_last:
                        epiq.append((c, po, pl))
                        del acc[c]

                for c, pair, lo, kind, is_last in units:
                    if epiq:
                        emit_epilogue()
                    qsl0 = c * CHUNK
                    st = stp.tile([D, 2, CHUNK], F32, tag="st", name="st")
                    if c == 0:
                        # bf16 ST per tile, full width (mask covers prefix)
                        for i, t in enumerate(pair):
                            nc.tensor.matmul(
                                st[:, i, :],
                                kt16[:, t * 128:(t + 1) * 128],
                                qt16,
                                start=True, stop=True, skip_group_check=True)
                            nc.tensor.matmul(
                                st[:, i, 0:128 * (t + 1)], ident,
                                msk[:, t * CHUNK:t * CHUNK + 128 * (t + 1)],
                                start=False, stop=True, skip_group_check=True)
                        pt = pt16s[npt16[0] % 2]
                        npt16[0] += 1
                        nc.scalar.activation(
                            pt[:, :, :], st[:, :, :],
                            mybir.ActivationFunctionType.Exp,
                            bias=bias_e, scale=EXP_SCALE)
                    else:
                        for i, t in enumerate(pair):
                            nc.tensor.matmul(
                                st[:, i, lo:],
                                ktp[:, :, t * 128:(t + 1) * 128],
                                qtp[:, :, qsl0 + lo:qsl0 + CHUNK],
                                start=True, stop=True,
                                perf_mode=DR, skip_group_check=True)
                            if kind == "diag":
                                m = t - 4 * c
                                nc.tensor.matmul(
                                    st[:, i, 0:128 * (m + 1)], ident,
                                    msk[:, m * CHUNK:m * CHUNK + 128 * (m + 1)],
                                    start=False, stop=True,
                                    skip_group_check=True)
                        pt = pt8s[npt[0] % 3]
                        npt[0] += 1
                        ntrick[0] += 1
                        if ntrick[0] % 2 == 0:
                            nc.vector.tensor_scalar(
                                out=pt[:, :, lo:], in0=st[:, :, lo:],
                                scalar1=BITS_B, scalar2=0.0,
                                op0=mybir.AluOpType.add,
                                op1=mybir.AluOpType.max)
                        else:
                            nc.scalar.activation(
                                pt[:, :, lo:], st[:, :, lo:],
                                mybir.ActivationFunctionType.Relu,
                                bias=bias_b, scale=1.0)
                    prev, pending = pending, (c, pair, lo, kind, is_last, pt)
                    if prev is not None:
                        emit_av(prev)
                emit_av(pending)
                while epiq:
                    emit_epilogue()
                nc.sync.dma_start(out=lv_d[:, :], in_=lb)
            stp_cm.__exit__(None, None, None)

    if legalize:
        _legalize_multiwaits(nc)
    nc.finalize()
    return nc


def _legalize_multiwaits(nc):
    """Hardware instruction structs in this walrus build accept at most ONE
    sync wait; hoist extra waits onto same-engine NoOps."""
    for fn in nc.m.functions:
        for blk in fn.blocks:
            insts = blk.instructions
            out = []
            for inst in insts:
                si = inst.sync_info
                if si is not None and si.on_wait and len(si.on_wait) >= 2:
                    waits = list(si.on_wait)
                    for w in waits[:-1]:
                        out.append(mybir.InstNoOp(
                            name=nc.get_next_instruction_name(),
                            engine=inst.engine,
                            bass_nofuse=True,
                            sync_info=mybir.SyncInfo(
                                on_wait=[w], on_update=[]),
                        ))
                    inst.sync_info = mybir.SyncInfo(
                        on_wait=[waits[-1]],
                        on_update=list(si.on_update or []))
                out.append(inst)
            insts[:] = out


_NC_CACHE = {}


def get_nc(legalize=True):
    key = ("nc", legalize)
    if key not in _NC_CACHE:
        _NC_CACHE[key] = build_nc(legalize)
    return _NC_CACHE[key]


def make_core_inputs(x, Wq, bq, Wk, bk, Wv, bv):
    """Per-core input maps. bk dropped (softmax invariance); bv host-side.
    Wq/bq folded with attention 1/sqrt(D) AND the bit-trick score scale."""
    s = BITS_SCALE / math.sqrt(D)
    wq_s = (np.asarray(Wq, np.float32) * s).astype(ml_dtypes.bfloat16)
    bq_s = (np.asarray(bq, np.float32) * s).astype(np.float32)
    wk = np.asarray(Wk, np.float32).astype(ml_dtypes.bfloat16)
    wv = np.asarray(Wv, np.float32).astype(ml_dtypes.bfloat16)

    qp = np.arange(CHUNK)[None, :]
    kk = np.arange(128)[:, None]
    msk = np.stack(
        [np.where(qp >= 128 * m + kk, 0.0, NEG) for m in range(4)]
    ).astype(ml_dtypes.bfloat16).reshape(4, D, CHUNK)
    ident = np.eye(D).astype(ml_dtypes.bfloat16)

    x = np.asarray(x, dtype=np.float32)
    in_maps = []
    for core in range(8):
        b, h = core // 2, core % 2
        xb = x[b]
        tri = xb[h * HALF:(h + 1) * HALF]           # [2048, 128]
        rect_q = xb[HALF:]                          # [2048, 128]
        rect_kv = xb[h * 1024:(h + 1) * 1024]       # [1024, 128]
        xt = np.ascontiguousarray(
            np.concatenate([tri, rect_q, rect_kv], axis=0).T
        ).astype(ml_dtypes.bfloat16)                # [128, 5120]
        in_maps.append({
            "xT": xt, "Wqs": wq_s, "Wk": wk, "Wv": wv, "bqs": bq_s,
            "msk": msk, "ident": ident,
        })
    return in_maps


def merge_outputs(results, bv):
    """Gather per-core (oT, lv) into the full [B, T, D] output."""
    bv = np.asarray(bv, dtype=np.float32)
    out = np.empty((B, T, D), np.float32)
    for b in range(B):
        lo, hi = results[2 * b], results[2 * b + 1]
        O = np.zeros((T, D), np.float64)
        L = np.zeros(T, np.float64)
        O[:HALF] += lo["oT"][:, :HALF].T
        L[:HALF] += lo["lv"][0:4].ravel()
        O[HALF:] += hi["oT"][:, :HALF].T
        L[HALF:] += hi["lv"][0:4].ravel()
        O[HALF:] += lo["oT"][:, HALF:].T
        L[HALF:] += lo["lv"][4:8].ravel()
        O[HALF:] += hi["oT"][:, HALF:].T
        L[HALF:] += hi["lv"][4:8].ravel()
        out[b] = (O / L[:, None]).astype(np.float32) + bv
    return out


def run_per_core(nc, in_maps, threads=True):
    """Run the same single-core program on each NeuronCore with its own
    inputs (axon tunnel: per-core dispatch; no collectives)."""
    import jax
    from concourse import bass2jax

    devices = jax.devices()[:len(in_maps)]

    def one(i):
        with jax.default_device(devices[i]):
            return bass2jax.run_bass_via_pjrt(nc, [in_maps[i]], n_cores=1)[0]

    if threads:
        from concurrent.futures import ThreadPoolExecutor
        first = one(0)
        with ThreadPoolExecutor(max_workers=7) as ex:
            rest = list(ex.map(one, range(1, len(in_maps))))
        return [first] + rest
    return [one(i) for i in range(len(in_maps))]


def kernel(x, Wq, bq, Wk, bk, Wv, bv, _trace=False):
    from concourse.bass_utils import axon_active, run_bass_kernel_spmd

    nc = get_nc()
    in_maps = make_core_inputs(x, Wq, bq, Wk, bk, Wv, bv)
    if axon_active():
        results = run_per_core(nc, in_maps)
    else:
        res = run_bass_kernel_spmd(nc, in_maps, list(range(8)), trace=_trace)
        kernel.last_result = res
        results = res.results
    out = merge_outputs(results, bv)
    return out
